# revision 24
# baseline (speedup 1.0000x reference)
"""Trainium2 Bass kernel for nn_DiffusionBlock: 20 steps of a 5-point
reflect-padded diffusion stencil on (16, 1, 1024, 1024) fp32.

The step operator is linear/separable: X <- a*X + Av X + X Aw^T with Av/Aw
1D reflect-BC neighbor-sum operators, diagonalized analytically by the DCT-I
basis v_k[i] = cos(pi*k*i/(N-1)). The T-step result is the spectral map
Y = F [ M * (E^T X E) ] F^T with M_ij = (a + lv_i + lw_j)^T.

Because the eigenvectors have exact parity (v_k[N-1-i] = (-1)^k v_k[i]),
the transforms split into symmetric/antisymmetric halves: the host folds X
into 4 parity quadrants (512x512), the device runs 4 independent half-size
spectral pipelines (halving all matmul work), and the host recombines the
output quadrants. Mid-band mask blocks decay to 0 after 20 steps and their
stage-4 matmuls are skipped entirely.

Data-parallel over batch: 2 images/core on 8 NeuronCores; fp32r matmuls
(full PE rate for 4-byte data); everything SBUF-resident per image.
"""

import sys

import numpy as np

if "/opt/trn_rl_repo" not in sys.path:
    sys.path.insert(0, "/opt/trn_rl_repo")

import concourse.bass as bass  # noqa: E402
import concourse.tile as tile  # noqa: E402
from concourse import bacc, mybir  # noqa: E402
from concourse.bass_utils import run_bass_kernel_spmd  # noqa: E402

N = 1024
H = 512           # half size after parity fold
P = 128
KCH = H // P      # 4 contraction chunks per quadrant stage
QF = H * KCH      # 2048: free size of one chunked 512x512 quadrant matrix
F = 4 * QF        # 8192: all four quadrants
NCORES = 8
IMGS_PER_CORE = 2
SPARSE_TH = 1e-5

_BASS_CACHE = {}
_MAT_CACHE = {}


def _chunk5(a):
    """(512, 512) -> (128, 2048); chunk k holds rows [128k, 128k+128)."""
    return np.ascontiguousarray(
        a.reshape(KCH, P, H).transpose(1, 0, 2).reshape(P, QF))


def _unchunk5(t):
    return np.ascontiguousarray(
        t.reshape(P, KCH, H).transpose(1, 0, 2).reshape(H, H))


def _build_specs(weight, time_steps):
    key = (weight.tobytes(), int(time_steps))
    if key in _MAT_CACHE:
        return _MAT_CACHE[key]
    w = np.asarray(weight, dtype=np.float64).reshape(3, 3)
    assert max(abs(w[0, 0]), abs(w[0, 2]), abs(w[2, 0]), abs(w[2, 2])) < 1e-12
    assert abs(w[0, 1] - w[2, 1]) < 1e-12 and abs(w[1, 0] - w[1, 2]) < 1e-12
    a_c = w[1, 1]
    k = np.arange(N)
    i = np.arange(N)
    lam = np.cos(np.pi * k / (N - 1))
    V = np.cos(np.pi * np.outer(i, k) / (N - 1))
    d = np.ones(N)
    d[0] = 0.5
    d[-1] = 0.5
    wn = np.sqrt((d[:, None] * V * V).sum(axis=0))
    E = (d[:, None] * V) / wn[None, :]
    Fm = V / wn[None, :]
    lv = (w[0, 1] + w[2, 1]) * lam
    lw = (w[1, 0] + w[1, 2]) * lam
    M = (a_c + lv[:, None] + lw[None, :]) ** int(time_steps)

    sym = np.arange(0, N, 2)
    anti = np.arange(1, N, 2)
    halves = (sym, anti)
    # eh: [Es | Ea], fh: [Fts | Fta] (Ft = F-half transposed: [j, r])
    eh = np.concatenate([_chunk5(E[:H, hv]) for hv in halves], axis=1)
    fh = np.concatenate([_chunk5(np.ascontiguousarray(Fm[:H, hv].T))
                         for hv in halves], axis=1)
    mq_list = []
    zero_blk = []   # [q][k][m] stage-4 skippable blocks
    for ri in range(2):
        for gi in range(2):
            Mq = M[np.ix_(halves[ri], halves[gi])]
            mq_list.append(_chunk5(Mq))
            zb = tuple(
                tuple(bool(np.max(np.abs(Mq[P * kk: P * (kk + 1),
                                            P * mm: P * (mm + 1)])) < SPARSE_TH)
                      for mm in range(KCH))
                for kk in range(KCH))
            for mm in range(KCH):
                assert not all(zb[kk][mm] for kk in range(KCH))
            zero_blk.append(zb)
    mq = np.concatenate(mq_list, axis=1)
    # S2 live column range per (quadrant, j1-row-tile): columns outside it are
    # exactly the stage-4-skipped blocks, so they are never read. Keep
    # N >= 256 (fp32r full-rate threshold).
    col_ranges = []
    for q in range(4):
        zb = zero_blk[q]
        cr = []
        for m in range(KCH):
            live = [mm for mm in range(KCH) if not zb[m][mm]]
            lo, hi = min(live) * P, (max(live) + 1) * P
            if hi - lo < 256:
                lo = min(lo, H - 256)
                hi = lo + 256
            cr.append((lo, hi))
        col_ranges.append(tuple(cr))
    out = (eh.astype(np.float32), fh.astype(np.float32),
           mq.astype(np.float32), tuple(zero_blk), tuple(col_ranges))
    _MAT_CACHE[key] = out
    return out


def _fold_image(img):
    """(1024, 1024) f32 -> (128, 8192) quadrant-folded chunk layout."""
    a = img.astype(np.float32)
    xp = a[:H] + a[N - 1:H - 1:-1]
    xm = a[:H] - a[N - 1:H - 1:-1]
    qs = []
    for xr in (xp, xm):
        qs.append(xr[:, :H] + xr[:, N - 1:H - 1:-1])
        qs.append(xr[:, :H] - xr[:, N - 1:H - 1:-1])
    return np.concatenate([_chunk5(q) for q in qs], axis=1)


def _unfold_image(yq):
    """(128, 8192) f32 quadrant outputs -> (1024, 1024)."""
    Qs = [_unchunk5(yq[:, QF * q: QF * (q + 1)]).astype(np.float64)
          for q in range(4)]
    Ypp, Ypm, Ymp, Ymm = Qs
    Y = np.empty((N, N), dtype=np.float32)
    Y[:H, :H] = Ypp + Ypm + Ymp + Ymm
    Y[:H, H:] = (Ypp - Ypm + Ymp - Ymm)[:, ::-1]
    Y[H:, :H] = (Ypp + Ypm - Ymp - Ymm)[::-1, :]
    Y[H:, H:] = (Ypp - Ypm - Ymp + Ymm)[::-1, ::-1]
    return Y


def _stage(nc, ppool, in_t, const_t, const_half, out_t, q, mask_t=None,
           skip_blocks=None, col_ranges=None, evac_engine="scalar"):
    """One quadrant stage: out[m-tile] += in_slice^T @ const, PSUM-accumulated
    over live k-chunks, then drained (optionally mask-multiplied).
    col_ranges[m] = (lo, hi) trims the output/moving columns to the live
    mask range (unwritten columns are never read downstream)."""
    qb = QF * q
    cb = QF * const_half
    for m in range(KCH):
        live_k = [kk for kk in range(KCH)
                  if not (skip_blocks and skip_blocks[kk][m])]
        lo, hi = col_ranges[m] if col_ranges else (0, H)
        ps = ppool.tile([P, H], mybir.dt.float32, tag="ps")
        for kk in live_k:
            nc.tensor.matmul(
                out=ps[:, lo:hi],
                lhsT=in_t[:, qb + H * kk + P * m: qb + H * kk + P * (m + 1)],
                rhs=const_t[:, cb + H * kk + lo: cb + H * kk + hi],
                start=(kk == live_k[0]),
                stop=(kk == live_k[-1]),
            )
        out_ap = out_t[:, qb + H * m + lo: qb + H * m + hi]
        if mask_t is not None:
            nc.vector.tensor_tensor(
                out=out_ap, in0=ps[:, lo:hi],
                in1=mask_t[:, qb + H * m + lo: qb + H * m + hi],
                op=mybir.AluOpType.mult)
        elif evac_engine == "scalar":
            nc.scalar.copy(out=out_ap, in_=ps[:, lo:hi])
        else:
            nc.vector.tensor_copy(out=out_ap, in_=ps[:, lo:hi])


def _build_bass(zero_blk, col_ranges):
    cache_key = ("quad", zero_blk, col_ranges)
    if cache_key in _BASS_CACHE:
        return _BASS_CACHE[cache_key]
    nc = bacc.Bacc("TRN2", target_bir_lowering=False, debug=False,
                   num_devices=NCORES)
    f32 = mybir.dt.float32
    f32r = mybir.dt.float32r
    xq_d = nc.dram_tensor("xq", [IMGS_PER_CORE, P, F], f32r,
                          kind="ExternalInput").ap()
    eh_d = nc.dram_tensor("eh", [P, 2 * QF], f32r, kind="ExternalInput").ap()
    fh_d = nc.dram_tensor("fh", [P, 2 * QF], f32r, kind="ExternalInput").ap()
    mq_d = nc.dram_tensor("mq", [P, F], f32, kind="ExternalInput").ap()
    yq_d = nc.dram_tensor("yq", [IMGS_PER_CORE, P, F], f32,
                          kind="ExternalOutput").ap()

    # quadrant q = 2*rho + gam; stage constants per quadrant:
    RHO = [0, 0, 1, 1]
    GAM = [0, 1, 0, 1]

    with tile.TileContext(nc) as tc:
        with tc.tile_pool(name="const", bufs=1) as cpool, \
             tc.tile_pool(name="data", bufs=1) as dpool, \
             tc.tile_pool(name="psum", bufs=8, space="PSUM") as ppool:
            eh_t = cpool.tile([P, 2 * QF], f32r, tag="eh")
            fh_t = cpool.tile([P, 2 * QF], f32r, tag="fh")
            mq_t = cpool.tile([P, F], f32, tag="mq")
            # (a HAM-warmup matmul block was tried here and removed: PE-queue
            # ordering made it delay the real stream more than the ~3us
            # cold-clock penalty it recovers)
            # DMA order = first-use order: X(img0)+E now, mask soon, Ft later;
            # the very first pieces are small so S1 can issue early
            xa0 = dpool.tile([P, F], f32r, tag="bufA")
            nc.sync.dma_start(out=xa0[:, 0:512], in_=xq_d[0, :, 0:512])
            nc.sync.dma_start(out=eh_t[:, 0:512], in_=eh_d[:, 0:512])
            nc.sync.dma_start(out=xa0[:, 512:1024], in_=xq_d[0, :, 512:1024])
            nc.sync.dma_start(out=eh_t[:, 512:1024], in_=eh_d[:, 512:1024])
            for c in range(1, 8):
                s = slice(1024 * c, 1024 * (c + 1))
                nc.sync.dma_start(out=xa0[:, s], in_=xq_d[0, :, s])
                if c < 4:
                    s2 = slice(1024 * c, 1024 * (c + 1))
                    nc.sync.dma_start(out=eh_t[:, s2], in_=eh_d[:, s2])
            for c in range(8):
                s = slice(1024 * c, 1024 * (c + 1))
                nc.sync.dma_start(out=mq_t[:, s], in_=mq_d[:, s])
            for c in range(4):
                s = slice(1024 * c, 1024 * (c + 1))
                nc.sync.dma_start(out=fh_t[:, s], in_=fh_d[:, s])

            for img in range(IMGS_PER_CORE):
                if img == 0:
                    xa = xa0
                else:
                    xa = dpool.tile([P, F], f32r, tag="bufA")
                    for c in range(8):
                        s = slice(1024 * c, 1024 * (c + 1))
                        nc.sync.dma_start(out=xa[:, s], in_=xq_d[img, :, s])

                wb = dpool.tile([P, F], f32r, tag="bufB")
                for q in range(4):      # S1: W = X^T E_rho
                    _stage(nc, ppool, xa, eh_t, RHO[q], wb, q)
                ga = dpool.tile([P, F], f32r, tag="bufA")
                for q in range(4):      # S2+S3: G = M * (W^T E_gam), trimmed
                    _stage(nc, ppool, wb, eh_t, GAM[q], ga, q, mask_t=mq_t,
                           col_ranges=col_ranges[q])
                hb = dpool.tile([P, F], f32r, tag="bufB")
                for q in range(4):      # S4: H = G^T Ft_rho (sparse)
                    _stage(nc, ppool, ga, fh_t, RHO[q], hb, q,
                           skip_blocks=zero_blk[q])
                yc = dpool.tile([P, F], f32, tag="bufC")
                for q in range(4):      # S5: Y = H^T Ft_gam
                    _stage(nc, ppool, hb, fh_t, GAM[q], yc, q)
                    for m in range(KCH):
                        s = slice(QF * q + H * m, QF * q + H * (m + 1))
                        nc.sync.dma_start(out=yq_d[img, :, s], in_=yc[:, s])

    nc.compile()
    _BASS_CACHE[cache_key] = nc
    return nc


def kernel(x, weight, time_steps, **_ignored):
    x = np.asarray(x, dtype=np.float32)
    weight = np.asarray(weight, dtype=np.float32)
    eh, fh, mq, zero_blk, col_ranges = _build_specs(weight, time_steps)
    nc = _build_bass(zero_blk, col_ranges)

    b = x.shape[0]
    assert b == NCORES * IMGS_PER_CORE and x.shape[-2:] == (N, N)
    in_maps = []
    for c in range(NCORES):
        xq = np.stack([_fold_image(x[c * IMGS_PER_CORE + i, 0])
                       for i in range(IMGS_PER_CORE)])
        in_maps.append({"xq": xq, "eh": eh, "fh": fh, "mq": mq})

    res = run_bass_kernel_spmd(nc, in_maps, core_ids=list(range(NCORES)))
    _BASS_CACHE["last_results"] = res

    out = np.empty((b, 1, N, N), dtype=np.float32)
    for c in range(NCORES):
        ys = res.results[c]["yq"]
        for i in range(IMGS_PER_CORE):
            out[c * IMGS_PER_CORE + i, 0] = _unfold_image(ys[i])
    return out
